# revision 11
# baseline (speedup 1.0000x reference)
"""Chamfer-augmented kernel for Trainium2 (8 NeuronCores, data-parallel over batch).

For each batch b and each grid sample s:
    mins[s]  = min_j ||grid_s - pred_j||
    mins2[s] = min_j ||grid_s - gt_j||
    out[b]   = mean_s |mins - mins2|

Per-core algorithm (batch b on core b):
  PSUM holds d^2(s,j) = x_s^2 + q_j - 2 x_s . y_j directly: a single K=21 bf16
  matmul per 512-col chunk using exact Karatsuba splits (x = xh+xl, y' = -2y =
  yh+yl, q = qh+ql per coordinate, x^2 = x2h+x2m+x2l):
    lhsT rows: [xh]*3 [xh]*3 [xl]*3 [xl]*3 [1]*6 [x2h x2m x2l]
    rhs  rows: [yh]*3 [yl]*3 [yh]*3 [yl]*3 [qh]*3 [ql]*3 [1]*3
  Evacuation never materializes the distance matrix: per m-tile (128 samples),
  8192 columns stream through an 8-bank PSUM ring as two 2048-col groups that
  ScalarE converts to f16 (CC) and four 1024-col groups that VectorE consumes
  with fused running-min scans:
    tensor_tensor_scan(out, data0=PSUM_f32, data1=CC_f16, init=chain,
                       op0=min, op1=min)
  Each scan first-touches 1 PSUM + 1 CC element per cycle, and the chain's
  initial value threads the running min across the four scans, so the m-tile
  min falls out of the last scan's final column with no separate fold tree.
"""

import os

import numpy as np

import concourse.bass as bass
import concourse.tile as tile
from concourse import bacc, mybir, bass_utils

F32 = mybir.dt.float32
BF16 = mybir.dt.bfloat16
F16 = mybir.dt.float16
AX = mybir.AxisListType
OP = mybir.AluOpType
AF = mybir.ActivationFunctionType

BS = 8
S = 2048          # n_samples (grid points)
J = 8192          # n_points (preds/gts)
NM = S // 128     # 16 m-tiles
PACK = 8          # prep packing for rhs: [3*PACK, J/PACK]
JP = J // PACK    # 1024
GPACK = 8         # prep packing for grid: [3*GPACK, S/GPACK]
SP = S // GPACK   # 256

# lhsT/rhs row layout (K = 24)
#   rows 0-2   lhsT xh_c        rhs yh_c
#   rows 3-5   lhsT xh_c        rhs yl_c
#   rows 6-8   lhsT xl_c        rhs yh_c
#   rows 9-11  lhsT xl_c        rhs yl_c
#   rows 12-14 lhsT ones        rhs qh_c
#   rows 15-17 lhsT ones        rhs ql_c
#   rows 18-23 lhsT gqh_c/gql_c rhs ones     (x^2 = sum_c g_c^2 via contraction)
K = 24


def _build_rhs(nc, sb, pts_dram, name, dma):
    """Load one point set (packed [24, 1024] f32) and build the [21, J] bf16 rhs."""
    Y = sb.tile([3 * PACK, JP], F32, tag=f"y_{name}")
    nc.sync.dma_start(Y[:], pts_dram)
    # q = y^2 per coordinate (ScalarE), yh = bf16(-2y) (ScalarE)
    SQ = sb.tile([3 * PACK, JP], F32, tag=f"sq_{name}")
    nc.scalar.activation(SQ[:], Y[:], AF.Square)
    YH = sb.tile([3 * PACK, JP], BF16, tag=f"yh_{name}")
    nc.scalar.activation(YH[:], Y[:], AF.Copy, scale=-2.0)
    # yl = (-2y) - yh (VectorE), qh = bf16(q) (ScalarE), ql = q - qh (VectorE)
    YL = sb.tile([3 * PACK, JP], BF16, tag=f"yl_{name}")
    nc.vector.scalar_tensor_tensor(YL[:], Y[:], -2.0, YH[:], op0=OP.mult, op1=OP.subtract)
    QH = sb.tile([3 * PACK, JP], BF16, tag=f"qh_{name}")
    nc.scalar.activation(QH[:], SQ[:], AF.Copy)
    QL = sb.tile([3 * PACK, JP], BF16, tag=f"ql_{name}")
    nc.vector.tensor_tensor(QL[:], SQ[:], QH[:], op=OP.subtract)
    ONESJ = sb.tile([3 * PACK, JP], BF16, tag=f"onesj_{name}")
    nc.gpsimd.memset(ONESJ[:], 1.0)

    RH = sb.tile([K, J], BF16, tag=f"rh_{name}")
    # packed [24, 1024] -> [3, 8192] row groups; AP iteration orders match.
    # ScalarE-sourced rows first so VectorE-dependent rows don't head-of-line
    # block the in-order DGE queue.
    for r0, src in ((18, ONESJ), (21, ONESJ), (0, YH), (6, YH), (12, QH)):
        dma(RH[r0:r0 + 3, :], src[:])
    for r0, src in ((3, YL), (9, YL), (15, QL)):
        nc.gpsimd.dma_start(RH[r0:r0 + 3, :], src[:])
    return RH


def _build_lhs(nc, sb, grid_dram):
    """Build the [24, S] bf16 lhsT from the packed grid [24, 256]."""
    GP = sb.tile([3 * GPACK, SP], F32, tag="gp")
    nc.sync.dma_start(GP[:], grid_dram)

    XH = sb.tile([3 * GPACK, SP], BF16, tag="xh")
    nc.scalar.activation(XH[:], GP[:], AF.Copy)
    XL = sb.tile([3 * GPACK, SP], BF16, tag="xl")
    nc.vector.tensor_tensor(XL[:], GP[:], XH[:], op=OP.subtract)
    # per-coord squares of the grid, split to bf16 pairs (x^2 via contraction)
    SQG = sb.tile([3 * GPACK, SP], F32, tag="sqg")
    nc.vector.tensor_tensor(SQG[:], GP[:], GP[:], op=OP.mult)
    GQH = sb.tile([3 * GPACK, SP], BF16, tag="gqh")
    nc.scalar.activation(GQH[:], SQG[:], AF.Copy)
    GQL = sb.tile([3 * GPACK, SP], BF16, tag="gql")
    nc.vector.tensor_tensor(GQL[:], SQG[:], GQH[:], op=OP.subtract)
    ONESS = sb.tile([3 * GPACK, SP], BF16, tag="oness")
    nc.gpsimd.memset(ONESS[:], 1.0)

    LH = sb.tile([K, S], BF16, tag="lh")
    for r0, src in ((0, XH), (6, XL), (12, ONESS), (18, GQH)):
        nc.sync.dma_start(LH[r0:r0 + 3, :], src[:])
    for r0, src in ((3, XH), (9, XL), (15, ONESS), (21, GQL)):
        nc.gpsimd.dma_start(LH[r0:r0 + 3, :], src[:])
    return LH


def _mtile(nc, wk, ps_a, ps_s, LH, RH, MINS, INF, m):
    """One m-tile: 4 act groups of 1024 (ScalarE -> f16 CC) and 4 independent
    1024-col running-min scans (VectorE) pairing fresh PSUM with CC.
    Both PSUM tags are double-buffered (8 banks total) so each group's
    matmuls prefill while the previous group is consumed."""
    if True:
        LHm = LH[:, m * 128:(m + 1) * 128]
        OB = wk.tile([128, 4096], F16, tag="so")
        for u in range(4):  # unit = [act 1024 | scan 1024], scans independent
            PA = ps_a.tile([128, 1024], F32, tag="pa")
            base = u * 2048
            for t in range(2):
                nc.tensor.matmul(PA[:, t * 512:(t + 1) * 512], LHm,
                                 RH[:, base + t * 512:base + (t + 1) * 512],
                                 start=True, stop=True)
            CC = wk.tile([128, 1024], F16, tag="cc", bufs=4)
            nc.scalar.activation(CC[:], PA[:], AF.Copy)
            PS = ps_s.tile([128, 1024], F32, tag="psc")
            for t in range(2):
                nc.tensor.matmul(PS[:, t * 512:(t + 1) * 512], LHm,
                                 RH[:, base + 1024 + t * 512:base + 1024 + (t + 1) * 512],
                                 start=True, stop=True)
            nc.vector.tensor_tensor_scan(OB[:, u * 1024:(u + 1) * 1024], PS[:], CC[:],
                                         INF[:], op0=OP.min, op1=OP.min)
        # m-tile min = min over the 4 independent scans' final columns
        nc.vector.tensor_reduce(MINS[:, m:m + 1], OB[:, 1023::1024], axis=AX.X, op=OP.min)


def _build_module():
    nc = bacc.Bacc("TRN2", target_bir_lowering=False, debug=False, num_devices=BS)
    grid_p = nc.dram_tensor("grid_p", [3 * GPACK, SP], F32, kind="ExternalInput").ap()
    preds_p = nc.dram_tensor("preds_p", [3 * PACK, JP], F32, kind="ExternalInput").ap()
    gts_p = nc.dram_tensor("gts_p", [3 * PACK, JP], F32, kind="ExternalInput").ap()
    out_d = nc.dram_tensor("out", [1, 1], F32, kind="ExternalOutput").ap()

    with tile.TileContext(nc) as tc:
        with tc.tile_pool(name="sb", bufs=1) as sb, \
             tc.tile_pool(name="wk", bufs=2) as wk, \
             tc.tile_pool(name="ps_a", bufs=2, space="PSUM") as ps_a, \
             tc.tile_pool(name="ps_s", bufs=2, space="PSUM") as ps_s:
            # PE warmup: keep the PE p-state ramp going before the real
            # matmuls arrive (dep-free dummy matmuls on a memset tile).
            DUMW = sb.tile([3 * GPACK, 512], BF16, tag="dumw")
            nc.gpsimd.memset(DUMW[:], 0.0)
            for w in range(16):
                PWU = ps_a.tile([128, 1024], F32, tag="pa")
                nc.tensor.matmul(PWU[:, 0:512], DUMW[:, 0:128], DUMW[:],
                                 start=True, stop=True)
            LH = _build_lhs(nc, sb, grid_p)
            RHP = _build_rhs(nc, sb, preds_p, "p", nc.sync.dma_start)
            RHG = _build_rhs(nc, sb, gts_p, "g", nc.gpsimd.dma_start)

            INF = sb.tile([128, 1], F32, tag="inf")
            nc.vector.memset(INF[:], 3.0e38)

            MINS_P = sb.tile([128, NM], F32, tag="minsp")
            MINS_G = sb.tile([128, NM], F32, tag="minsg")

            # d = sqrt(max(d^2, eps)) with one Newton step; the two sets'
            # chains are interleaved so per-op sem gaps overlap.
            def _distances2(MP, MG):
                Ds = []
                for tag, MINS in (("dp", MP), ("dg", MG)):
                    D2 = sb.tile([128, NM], F32, tag=f"d2{tag}", name=f"d2{tag}")
                    Ds.append(D2)
                for D2, MINS in zip(Ds, (MP, MG)):
                    nc.vector.tensor_scalar_max(D2[:], MINS[:], 1e-12)
                D0s = []
                for tag in ("dp", "dg"):
                    D0 = sb.tile([128, NM], F32, tag=f"d0{tag}", name=f"d0{tag}")
                    D0s.append(D0)
                for D0, D2 in zip(D0s, Ds):
                    nc.scalar.activation(D0[:], D2[:], AF.Sqrt)
                Rs = []
                for tag in ("dp", "dg"):
                    R = sb.tile([128, NM], F32, tag=f"r{tag}", name=f"r{tag}")
                    Rs.append(R)
                for R, D0 in zip(Rs, D0s):
                    nc.vector.reciprocal(R[:], D0[:])
                D1s = []
                for tag in ("dp", "dg"):
                    D1 = sb.tile([128, NM], F32, tag=f"d1{tag}", name=f"d1{tag}")
                    D1s.append(D1)
                for D1, D2, R in zip(D1s, Ds, Rs):
                    nc.vector.tensor_tensor(D1[:], D2[:], R[:], op=OP.mult)
                for D1, D0 in zip(D1s, D0s):
                    nc.vector.tensor_tensor(D1[:], D1[:], D0[:], op=OP.add)
                for D1 in D1s:
                    nc.vector.tensor_scalar_mul(D1[:], D1[:], 0.5)
                return D1s

            for m in range(NM):
                _mtile(nc, wk, ps_a, ps_s, LH, RHP, MINS_P, INF, m)
                _mtile(nc, wk, ps_a, ps_s, LH, RHG, MINS_G, INF, m)
            DP, DG = _distances2(MINS_P, MINS_G)

            # mean_s |dp - dg|
            DIFF = sb.tile([128, NM], F32, tag="diff")
            nc.vector.tensor_tensor(DIFF[:], DP[:], DG[:], op=OP.subtract)
            SROW = sb.tile([128, 1], F32, tag="srow")
            nc.vector.tensor_reduce(SROW[:], DIFF[:], axis=AX.X, op=OP.add,
                                    apply_absolute_value=True)
            ONE32 = sb.tile([128, 1], F32, tag="one32")
            nc.vector.memset(ONE32[:], 1.0)
            PGX = ps_a.tile([128, 1024], F32, tag="pa")
            TOT = PGX[0:1, 0:1]
            nc.tensor.matmul(TOT, ONE32[:], SROW[:], start=True, stop=True)
            OUT = sb.tile([1, 1], F32, tag="outsb")
            nc.scalar.activation(OUT[:], TOT, AF.Copy, scale=1.0 / float(S))
            nc.sync.dma_start(out_d, OUT[:])
    nc.compile()
    return nc


_NC = None


def _get_nc():
    global _NC
    if _NC is None:
        _NC = _build_module()
    return _NC


def _in_maps(gts, preds, grid_points):
    maps = []
    for b in range(BS):
        g = np.ascontiguousarray(grid_points[b], np.float32)
        maps.append({
            "grid_p": np.ascontiguousarray(g.T.reshape(3 * GPACK, SP)),
            "preds_p": np.ascontiguousarray(preds[b], np.float32).T.reshape(3 * PACK, JP).copy(),
            "gts_p": np.ascontiguousarray(gts[b], np.float32).T.reshape(3 * PACK, JP).copy(),
        })
    return maps


def kernel(gts, preds, grid_points, _trace=False, _trace_kwargs=None):
    nc = _get_nc()
    res = bass_utils.run_bass_kernel_spmd(
        nc, _in_maps(gts, preds, grid_points), core_ids=list(range(BS)),
        trace=_trace, **(_trace_kwargs or {}))
    out = np.array([res.results[b]["out"][0, 0] for b in range(BS)], np.float32)
    if _trace:
        return out, res
    return out


# revision 12
# speedup vs baseline: 1.0254x; 1.0254x over previous
"""Chamfer-augmented kernel for Trainium2 (8 NeuronCores, data-parallel over batch).

For each batch b and each grid sample s:
    mins[s]  = min_j ||grid_s - pred_j||
    mins2[s] = min_j ||grid_s - gt_j||
    out[b]   = mean_s |mins - mins2|

Per-core algorithm (batch b on core b):
  PSUM holds d^2(s,j) = x_s^2 + q_j - 2 x_s . y_j directly: a single K=21 bf16
  matmul per 512-col chunk using exact Karatsuba splits (x = xh+xl, y' = -2y =
  yh+yl, q = qh+ql per coordinate, x^2 = x2h+x2m+x2l):
    lhsT rows: [xh]*3 [xh]*3 [xl]*3 [xl]*3 [1]*6 [x2h x2m x2l]
    rhs  rows: [yh]*3 [yl]*3 [yh]*3 [yl]*3 [qh]*3 [ql]*3 [1]*3
  Evacuation never materializes the distance matrix: per m-tile (128 samples),
  8192 columns stream through an 8-bank PSUM ring as two 2048-col groups that
  ScalarE converts to f16 (CC) and four 1024-col groups that VectorE consumes
  with fused running-min scans:
    tensor_tensor_scan(out, data0=PSUM_f32, data1=CC_f16, init=chain,
                       op0=min, op1=min)
  Each scan first-touches 1 PSUM + 1 CC element per cycle, and the chain's
  initial value threads the running min across the four scans, so the m-tile
  min falls out of the last scan's final column with no separate fold tree.
"""

import os

import numpy as np

import concourse.bass as bass
import concourse.tile as tile
from concourse import bacc, mybir, bass_utils

F32 = mybir.dt.float32
BF16 = mybir.dt.bfloat16
F16 = mybir.dt.float16
AX = mybir.AxisListType
OP = mybir.AluOpType
AF = mybir.ActivationFunctionType

BS = 8
S = 2048          # n_samples (grid points)
J = 8192          # n_points (preds/gts)
NM = S // 128     # 16 m-tiles
PACK = 8          # prep packing for rhs: [3*PACK, J/PACK]
JP = J // PACK    # 1024
GPACK = 8         # prep packing for grid: [3*GPACK, S/GPACK]
SP = S // GPACK   # 256

# lhsT/rhs row layout (K = 24)
#   rows 0-2   lhsT xh_c        rhs yh_c
#   rows 3-5   lhsT xh_c        rhs yl_c
#   rows 6-8   lhsT xl_c        rhs yh_c
#   rows 9-11  lhsT xl_c        rhs yl_c
#   rows 12-14 lhsT ones        rhs qh_c
#   rows 15-17 lhsT ones        rhs ql_c
#   rows 18-23 lhsT gqh_c/gql_c rhs ones     (x^2 = sum_c g_c^2 via contraction)
K = 24


def _build_rhs(nc, sb, pts_dram, name, dma):
    """Load one point set (packed [24, 1024] f32) and build the [21, J] bf16 rhs."""
    Y = sb.tile([3 * PACK, JP], F32, tag=f"y_{name}")
    nc.sync.dma_start(Y[:], pts_dram)
    # q = y^2 per coordinate (ScalarE), yh = bf16(-2y) (ScalarE)
    SQ = sb.tile([3 * PACK, JP], F32, tag=f"sq_{name}")
    nc.scalar.activation(SQ[:], Y[:], AF.Square)
    YH = sb.tile([3 * PACK, JP], BF16, tag=f"yh_{name}")
    nc.scalar.activation(YH[:], Y[:], AF.Copy, scale=-2.0)
    # yl = (-2y) - yh (VectorE), qh = bf16(q) (ScalarE), ql = q - qh (VectorE)
    YL = sb.tile([3 * PACK, JP], BF16, tag=f"yl_{name}")
    nc.vector.scalar_tensor_tensor(YL[:], Y[:], -2.0, YH[:], op0=OP.mult, op1=OP.subtract)
    QH = sb.tile([3 * PACK, JP], BF16, tag=f"qh_{name}")
    nc.scalar.activation(QH[:], SQ[:], AF.Copy)
    QL = sb.tile([3 * PACK, JP], BF16, tag=f"ql_{name}")
    nc.vector.tensor_tensor(QL[:], SQ[:], QH[:], op=OP.subtract)
    ONESJ = sb.tile([3 * PACK, JP], BF16, tag=f"onesj_{name}")
    nc.gpsimd.memset(ONESJ[:], 1.0)

    RH = sb.tile([K, J], BF16, tag=f"rh_{name}")
    # packed [24, 1024] -> [3, 8192] row groups; AP iteration orders match.
    # ScalarE-sourced rows first so VectorE-dependent rows don't head-of-line
    # block the in-order DGE queue.
    for r0, src in ((18, ONESJ), (21, ONESJ), (0, YH), (6, YH), (12, QH),
                    (3, YL), (9, YL), (15, QL)):
        dma(RH[r0:r0 + 3, :], src[:])
    return RH


def _build_lhs(nc, sb, grid_dram):
    """Build the [24, S] bf16 lhsT from the packed grid [24, 256]."""
    GP = sb.tile([3 * GPACK, SP], F32, tag="gp")
    nc.sync.dma_start(GP[:], grid_dram)

    XH = sb.tile([3 * GPACK, SP], BF16, tag="xh")
    nc.scalar.activation(XH[:], GP[:], AF.Copy)
    XL = sb.tile([3 * GPACK, SP], BF16, tag="xl")
    nc.vector.tensor_tensor(XL[:], GP[:], XH[:], op=OP.subtract)
    # per-coord squares of the grid, split to bf16 pairs (x^2 via contraction)
    SQG = sb.tile([3 * GPACK, SP], F32, tag="sqg")
    nc.vector.tensor_tensor(SQG[:], GP[:], GP[:], op=OP.mult)
    GQH = sb.tile([3 * GPACK, SP], BF16, tag="gqh")
    nc.scalar.activation(GQH[:], SQG[:], AF.Copy)
    GQL = sb.tile([3 * GPACK, SP], BF16, tag="gql")
    nc.vector.tensor_tensor(GQL[:], SQG[:], GQH[:], op=OP.subtract)
    ONESS = sb.tile([3 * GPACK, SP], BF16, tag="oness")
    nc.gpsimd.memset(ONESS[:], 1.0)

    LH = sb.tile([K, S], BF16, tag="lh")
    for r0, src in ((0, XH), (3, XH), (12, ONESS), (15, ONESS), (18, GQH),
                    (6, XL), (9, XL), (21, GQL)):
        nc.sync.dma_start(LH[r0:r0 + 3, :], src[:])
    return LH


def _mtile(nc, wk, ps_a, ps_s, LH, RH, MINS, INF, m):
    """One m-tile: 4 act groups of 1024 (ScalarE -> f16 CC) and 4 independent
    1024-col running-min scans (VectorE) pairing fresh PSUM with CC.
    Both PSUM tags are double-buffered (8 banks total) so each group's
    matmuls prefill while the previous group is consumed."""
    if True:
        LHm = LH[:, m * 128:(m + 1) * 128]
        OB = wk.tile([128, 4096], F16, tag="so")
        for u in range(4):  # unit = [act 1024 | scan 1024], scans independent
            PA = ps_a.tile([128, 1024], F32, tag="pa")
            base = u * 2048
            for t in range(2):
                nc.tensor.matmul(PA[:, t * 512:(t + 1) * 512], LHm,
                                 RH[:, base + t * 512:base + (t + 1) * 512],
                                 start=True, stop=True)
            CC = wk.tile([128, 1024], F16, tag="cc", bufs=4)
            nc.scalar.activation(CC[:], PA[:], AF.Copy)
            PS = ps_s.tile([128, 1024], F32, tag="psc")
            for t in range(2):
                nc.tensor.matmul(PS[:, t * 512:(t + 1) * 512], LHm,
                                 RH[:, base + 1024 + t * 512:base + 1024 + (t + 1) * 512],
                                 start=True, stop=True)
            nc.vector.tensor_tensor_scan(OB[:, u * 1024:(u + 1) * 1024], PS[:], CC[:],
                                         INF[:], op0=OP.min, op1=OP.min)
        # m-tile min = min over the 4 independent scans' final columns
        nc.vector.tensor_reduce(MINS[:, m:m + 1], OB[:, 1023::1024], axis=AX.X, op=OP.min)


def _build_module():
    nc = bacc.Bacc("TRN2", target_bir_lowering=False, debug=False, num_devices=BS)
    grid_p = nc.dram_tensor("grid_p", [3 * GPACK, SP], F32, kind="ExternalInput").ap()
    preds_p = nc.dram_tensor("preds_p", [3 * PACK, JP], F32, kind="ExternalInput").ap()
    gts_p = nc.dram_tensor("gts_p", [3 * PACK, JP], F32, kind="ExternalInput").ap()
    out_d = nc.dram_tensor("out", [1, 1], F32, kind="ExternalOutput").ap()

    with tile.TileContext(nc) as tc:
        with tc.tile_pool(name="sb", bufs=1) as sb, \
             tc.tile_pool(name="wk", bufs=2) as wk, \
             tc.tile_pool(name="ps_a", bufs=2, space="PSUM") as ps_a, \
             tc.tile_pool(name="ps_s", bufs=2, space="PSUM") as ps_s:
            # PE warmup: keep the PE p-state ramp going before the real
            # matmuls arrive (dep-free dummy matmuls on a memset tile).
            DUMW = sb.tile([3 * GPACK, 512], BF16, tag="dumw")
            nc.gpsimd.memset(DUMW[:], 0.0)
            for w in range(16):
                PWU = ps_a.tile([128, 1024], F32, tag="pa")
                nc.tensor.matmul(PWU[:, 0:512], DUMW[:, 0:128], DUMW[:],
                                 start=True, stop=True)
            LH = _build_lhs(nc, sb, grid_p)
            RHP = _build_rhs(nc, sb, preds_p, "p", nc.sync.dma_start)
            RHG = _build_rhs(nc, sb, gts_p, "g", nc.gpsimd.dma_start)

            INF = sb.tile([128, 1], F32, tag="inf")
            nc.vector.memset(INF[:], 3.0e38)

            MINS_P = sb.tile([128, NM], F32, tag="minsp")
            MINS_G = sb.tile([128, NM], F32, tag="minsg")

            # d = sqrt(max(d^2, eps)) with one Newton refinement step
            def _distances(MINS, tag):
                D2 = sb.tile([128, NM], F32, tag=f"d2{tag}", name=f"d2{tag}")
                nc.vector.tensor_scalar_max(D2[:], MINS[:], 1e-12)
                D0 = sb.tile([128, NM], F32, tag=f"d0{tag}", name=f"d0{tag}")
                nc.scalar.activation(D0[:], D2[:], AF.Sqrt)
                R = sb.tile([128, NM], F32, tag=f"r{tag}", name=f"r{tag}")
                nc.vector.reciprocal(R[:], D0[:])
                D1 = sb.tile([128, NM], F32, tag=f"d1{tag}", name=f"d1{tag}")
                nc.vector.tensor_tensor(D1[:], D2[:], R[:], op=OP.mult)
                nc.vector.tensor_tensor(D1[:], D1[:], D0[:], op=OP.add)
                nc.vector.tensor_scalar_mul(D1[:], D1[:], 0.5)
                return D1

            for m in range(NM):
                _mtile(nc, wk, ps_a, ps_s, LH, RHP, MINS_P, INF, m)
            DP = _distances(MINS_P, "dp")
            for m in range(NM):
                _mtile(nc, wk, ps_a, ps_s, LH, RHG, MINS_G, INF, m)
            DG = _distances(MINS_G, "dg")

            # mean_s |dp - dg|
            DIFF = sb.tile([128, NM], F32, tag="diff")
            nc.vector.tensor_tensor(DIFF[:], DP[:], DG[:], op=OP.subtract)
            SROW = sb.tile([128, 1], F32, tag="srow")
            nc.vector.tensor_reduce(SROW[:], DIFF[:], axis=AX.X, op=OP.add,
                                    apply_absolute_value=True)
            ONE32 = sb.tile([128, 1], F32, tag="one32")
            nc.vector.memset(ONE32[:], 1.0)
            PGX = ps_a.tile([128, 1024], F32, tag="pa")
            TOT = PGX[0:1, 0:1]
            nc.tensor.matmul(TOT, ONE32[:], SROW[:], start=True, stop=True)
            OUT = sb.tile([1, 1], F32, tag="outsb")
            nc.scalar.activation(OUT[:], TOT, AF.Copy, scale=1.0 / float(S))
            nc.sync.dma_start(out_d, OUT[:])
    nc.compile()
    return nc


_NC = None


def _get_nc():
    global _NC
    if _NC is None:
        _NC = _build_module()
    return _NC


def _in_maps(gts, preds, grid_points):
    maps = []
    for b in range(BS):
        g = np.ascontiguousarray(grid_points[b], np.float32)
        maps.append({
            "grid_p": np.ascontiguousarray(g.T.reshape(3 * GPACK, SP)),
            "preds_p": np.ascontiguousarray(preds[b], np.float32).T.reshape(3 * PACK, JP).copy(),
            "gts_p": np.ascontiguousarray(gts[b], np.float32).T.reshape(3 * PACK, JP).copy(),
        })
    return maps


def kernel(gts, preds, grid_points, _trace=False, _trace_kwargs=None):
    nc = _get_nc()
    res = bass_utils.run_bass_kernel_spmd(
        nc, _in_maps(gts, preds, grid_points), core_ids=list(range(BS)),
        trace=_trace, **(_trace_kwargs or {}))
    out = np.array([res.results[b]["out"][0, 0] for b in range(BS)], np.float32)
    if _trace:
        return out, res
    return out


# revision 13
# speedup vs baseline: 1.0475x; 1.0215x over previous
"""Chamfer-augmented kernel for Trainium2 (8 NeuronCores, data-parallel over batch).

For each batch b and each grid sample s:
    mins[s]  = min_j ||grid_s - pred_j||
    mins2[s] = min_j ||grid_s - gt_j||
    out[b]   = mean_s |mins - mins2|

Per-core algorithm (batch b on core b):
  PSUM holds d^2(s,j) = x_s^2 + q_j - 2 x_s . y_j directly: a single K=21 bf16
  matmul per 512-col chunk using exact Karatsuba splits (x = xh+xl, y' = -2y =
  yh+yl, q = qh+ql per coordinate, x^2 = x2h+x2m+x2l):
    lhsT rows: [xh]*3 [xh]*3 [xl]*3 [xl]*3 [1]*6 [x2h x2m x2l]
    rhs  rows: [yh]*3 [yl]*3 [yh]*3 [yl]*3 [qh]*3 [ql]*3 [1]*3
  Evacuation never materializes the distance matrix: per m-tile (128 samples),
  8192 columns stream through an 8-bank PSUM ring as two 2048-col groups that
  ScalarE converts to f16 (CC) and four 1024-col groups that VectorE consumes
  with fused running-min scans:
    tensor_tensor_scan(out, data0=PSUM_f32, data1=CC_f16, init=chain,
                       op0=min, op1=min)
  Each scan first-touches 1 PSUM + 1 CC element per cycle, and the chain's
  initial value threads the running min across the four scans, so the m-tile
  min falls out of the last scan's final column with no separate fold tree.
"""

import os

import numpy as np

import concourse.bass as bass
import concourse.tile as tile
from concourse import bacc, mybir, bass_utils

F32 = mybir.dt.float32
BF16 = mybir.dt.bfloat16
F16 = mybir.dt.float16
AX = mybir.AxisListType
OP = mybir.AluOpType
AF = mybir.ActivationFunctionType

BS = 8
S = 2048          # n_samples (grid points)
J = 8192          # n_points (preds/gts)
NM = S // 128     # 16 m-tiles
PACK = 8          # prep packing for rhs: [3*PACK, J/PACK]
JP = J // PACK    # 1024
GPACK = 8         # prep packing for grid: [3*GPACK, S/GPACK]
SP = S // GPACK   # 256

# lhsT/rhs row layout (K = 24)
#   rows 0-2   lhsT xh_c        rhs yh_c
#   rows 3-5   lhsT xh_c        rhs yl_c
#   rows 6-8   lhsT xl_c        rhs yh_c
#   rows 9-11  lhsT xl_c        rhs yl_c
#   rows 12-14 lhsT ones        rhs qh_c
#   rows 15-17 lhsT ones        rhs ql_c
#   rows 18-23 lhsT gqh_c/gql_c rhs ones     (x^2 = sum_c g_c^2 via contraction)
K = 24


def _build_rhs(nc, sb, pts_dram, name, dma):
    """Load one point set (packed [24, 1024] f32) and build the [21, J] bf16 rhs."""
    Y = sb.tile([3 * PACK, JP], F32, tag=f"y_{name}")
    nc.sync.dma_start(Y[:], pts_dram)
    # q = y^2 per coordinate (ScalarE), yh = bf16(-2y) (ScalarE)
    SQ = sb.tile([3 * PACK, JP], F32, tag=f"sq_{name}")
    nc.scalar.activation(SQ[:], Y[:], AF.Square)
    YH = sb.tile([3 * PACK, JP], BF16, tag=f"yh_{name}")
    nc.scalar.activation(YH[:], Y[:], AF.Copy, scale=-2.0)
    # yl = (-2y) - yh (VectorE), qh = bf16(q) (ScalarE), ql = q - qh (VectorE)
    YL = sb.tile([3 * PACK, JP], BF16, tag=f"yl_{name}")
    nc.vector.scalar_tensor_tensor(YL[:], Y[:], -2.0, YH[:], op0=OP.mult, op1=OP.subtract)
    QH = sb.tile([3 * PACK, JP], BF16, tag=f"qh_{name}")
    nc.scalar.activation(QH[:], SQ[:], AF.Copy)
    QL = sb.tile([3 * PACK, JP], BF16, tag=f"ql_{name}")
    nc.vector.tensor_tensor(QL[:], SQ[:], QH[:], op=OP.subtract)
    ONESJ = sb.tile([3 * PACK, JP], BF16, tag=f"onesj_{name}")
    nc.gpsimd.memset(ONESJ[:], 1.0)

    RH = sb.tile([K, J], BF16, tag=f"rh_{name}")
    # packed [24, 1024] -> [3, 8192] row groups; AP iteration orders match.
    # ScalarE-sourced rows first so VectorE-dependent rows don't head-of-line
    # block the in-order DGE queue.
    for r0, src in ((18, ONESJ), (21, ONESJ), (0, YH), (6, YH), (12, QH),
                    (3, YL), (9, YL), (15, QL)):
        dma(RH[r0:r0 + 3, :], src[:])
    return RH


def _build_lhs(nc, sb, grid_dram):
    """Build the [24, S] bf16 lhsT from the packed grid [24, 256]."""
    GP = sb.tile([3 * GPACK, SP], F32, tag="gp")
    nc.sync.dma_start(GP[:], grid_dram)

    XH = sb.tile([3 * GPACK, SP], BF16, tag="xh")
    nc.scalar.activation(XH[:], GP[:], AF.Copy)
    XL = sb.tile([3 * GPACK, SP], BF16, tag="xl")
    nc.vector.tensor_tensor(XL[:], GP[:], XH[:], op=OP.subtract)
    # per-coord squares of the grid, split to bf16 pairs (x^2 via contraction)
    SQG = sb.tile([3 * GPACK, SP], F32, tag="sqg")
    nc.vector.tensor_tensor(SQG[:], GP[:], GP[:], op=OP.mult)
    GQH = sb.tile([3 * GPACK, SP], BF16, tag="gqh")
    nc.scalar.activation(GQH[:], SQG[:], AF.Copy)
    GQL = sb.tile([3 * GPACK, SP], BF16, tag="gql")
    nc.vector.tensor_tensor(GQL[:], SQG[:], GQH[:], op=OP.subtract)
    ONESS = sb.tile([3 * GPACK, SP], BF16, tag="oness")
    nc.gpsimd.memset(ONESS[:], 1.0)

    LH = sb.tile([K, S], BF16, tag="lh")
    for r0, src in ((0, XH), (6, XL), (12, ONESS), (18, GQH)):
        nc.sync.dma_start(LH[r0:r0 + 3, :], src[:])
    for r0, src in ((3, XH), (9, XL), (15, ONESS), (21, GQL)):
        nc.gpsimd.dma_start(LH[r0:r0 + 3, :], src[:])
    return LH


def _mtile(nc, wk, ps_a, ps_s, LH, RH, MINS, INF, m):
    """One m-tile: 4 act groups of 1024 (ScalarE -> f16 CC) and 4 independent
    1024-col running-min scans (VectorE) pairing fresh PSUM with CC.
    Both PSUM tags are double-buffered (8 banks total) so each group's
    matmuls prefill while the previous group is consumed."""
    if True:
        LHm = LH[:, m * 128:(m + 1) * 128]
        OB = wk.tile([128, 4096], F16, tag="so")
        for u in range(4):  # unit = [act 1024 | scan 1024], scans independent
            PA = ps_a.tile([128, 1024], F32, tag="pa")
            base = u * 2048
            for t in range(2):
                nc.tensor.matmul(PA[:, t * 512:(t + 1) * 512], LHm,
                                 RH[:, base + t * 512:base + (t + 1) * 512],
                                 start=True, stop=True)
            CC = wk.tile([128, 1024], F16, tag="cc", bufs=4)
            nc.scalar.activation(CC[:], PA[:], AF.Copy)
            PS = ps_s.tile([128, 1024], F32, tag="psc")
            for t in range(2):
                nc.tensor.matmul(PS[:, t * 512:(t + 1) * 512], LHm,
                                 RH[:, base + 1024 + t * 512:base + 1024 + (t + 1) * 512],
                                 start=True, stop=True)
            nc.vector.tensor_tensor_scan(OB[:, u * 1024:(u + 1) * 1024], PS[:], CC[:],
                                         INF[:], op0=OP.min, op1=OP.min)
        # m-tile min = min over the 4 independent scans' final columns
        nc.vector.tensor_reduce(MINS[:, m:m + 1], OB[:, 1023::1024], axis=AX.X, op=OP.min)


def _build_module():
    nc = bacc.Bacc("TRN2", target_bir_lowering=False, debug=False, num_devices=BS)
    grid_p = nc.dram_tensor("grid_p", [3 * GPACK, SP], F32, kind="ExternalInput").ap()
    preds_p = nc.dram_tensor("preds_p", [3 * PACK, JP], F32, kind="ExternalInput").ap()
    gts_p = nc.dram_tensor("gts_p", [3 * PACK, JP], F32, kind="ExternalInput").ap()
    out_d = nc.dram_tensor("out", [1, 1], F32, kind="ExternalOutput").ap()

    with tile.TileContext(nc) as tc:
        with tc.tile_pool(name="sb", bufs=1) as sb, \
             tc.tile_pool(name="wk", bufs=2) as wk, \
             tc.tile_pool(name="ps_a", bufs=2, space="PSUM") as ps_a, \
             tc.tile_pool(name="ps_s", bufs=2, space="PSUM") as ps_s:
            LH = _build_lhs(nc, sb, grid_p)
            RHP = _build_rhs(nc, sb, preds_p, "p", nc.sync.dma_start)
            RHG = _build_rhs(nc, sb, gts_p, "g", nc.gpsimd.dma_start)

            INF = sb.tile([128, 1], F32, tag="inf")
            nc.vector.memset(INF[:], 3.0e38)

            MINS_P = sb.tile([128, NM], F32, tag="minsp")
            MINS_G = sb.tile([128, NM], F32, tag="minsg")

            # d = sqrt(max(d^2, eps)) with one Newton refinement step
            def _distances(MINS, tag):
                D2 = sb.tile([128, NM], F32, tag=f"d2{tag}", name=f"d2{tag}")
                nc.vector.tensor_scalar_max(D2[:], MINS[:], 1e-12)
                D0 = sb.tile([128, NM], F32, tag=f"d0{tag}", name=f"d0{tag}")
                nc.scalar.activation(D0[:], D2[:], AF.Sqrt)
                R = sb.tile([128, NM], F32, tag=f"r{tag}", name=f"r{tag}")
                nc.vector.reciprocal(R[:], D0[:])
                D1 = sb.tile([128, NM], F32, tag=f"d1{tag}", name=f"d1{tag}")
                nc.vector.tensor_tensor(D1[:], D2[:], R[:], op=OP.mult)
                nc.vector.tensor_tensor(D1[:], D1[:], D0[:], op=OP.add)
                nc.vector.tensor_scalar_mul(D1[:], D1[:], 0.5)
                return D1

            for m in range(NM):
                _mtile(nc, wk, ps_a, ps_s, LH, RHP, MINS_P, INF, m)
            DP = _distances(MINS_P, "dp")
            for m in range(NM):
                _mtile(nc, wk, ps_a, ps_s, LH, RHG, MINS_G, INF, m)
            DG = _distances(MINS_G, "dg")

            # mean_s |dp - dg|
            DIFF = sb.tile([128, NM], F32, tag="diff")
            nc.vector.tensor_tensor(DIFF[:], DP[:], DG[:], op=OP.subtract)
            SROW = sb.tile([128, 1], F32, tag="srow")
            nc.vector.tensor_reduce(SROW[:], DIFF[:], axis=AX.X, op=OP.add,
                                    apply_absolute_value=True)
            ONE32 = sb.tile([128, 1], F32, tag="one32")
            nc.vector.memset(ONE32[:], 1.0)
            PGX = ps_a.tile([128, 1024], F32, tag="pa")
            TOT = PGX[0:1, 0:1]
            nc.tensor.matmul(TOT, ONE32[:], SROW[:], start=True, stop=True)
            OUT = sb.tile([1, 1], F32, tag="outsb")
            nc.scalar.activation(OUT[:], TOT, AF.Copy, scale=1.0 / float(S))
            nc.sync.dma_start(out_d, OUT[:])
    nc.compile()
    return nc


_NC = None


def _get_nc():
    global _NC
    if _NC is None:
        _NC = _build_module()
    return _NC


def _in_maps(gts, preds, grid_points):
    maps = []
    for b in range(BS):
        g = np.ascontiguousarray(grid_points[b], np.float32)
        maps.append({
            "grid_p": np.ascontiguousarray(g.T.reshape(3 * GPACK, SP)),
            "preds_p": np.ascontiguousarray(preds[b], np.float32).T.reshape(3 * PACK, JP).copy(),
            "gts_p": np.ascontiguousarray(gts[b], np.float32).T.reshape(3 * PACK, JP).copy(),
        })
    return maps


def kernel(gts, preds, grid_points, _trace=False, _trace_kwargs=None):
    nc = _get_nc()
    res = bass_utils.run_bass_kernel_spmd(
        nc, _in_maps(gts, preds, grid_points), core_ids=list(range(BS)),
        trace=_trace, **(_trace_kwargs or {}))
    out = np.array([res.results[b]["out"][0, 0] for b in range(BS)], np.float32)
    if _trace:
        return out, res
    return out


# revision 14
# speedup vs baseline: 1.0544x; 1.0066x over previous
"""Chamfer-augmented kernel for Trainium2 (8 NeuronCores, data-parallel over batch).

For each batch b and each grid sample s:
    mins[s]  = min_j ||grid_s - pred_j||
    mins2[s] = min_j ||grid_s - gt_j||
    out[b]   = mean_s |mins - mins2|

Per-core algorithm (batch b on core b):
  PSUM holds d^2(s,j) = x_s^2 + q_j - 2 x_s . y_j directly: a single K=21 bf16
  matmul per 512-col chunk using exact Karatsuba splits (x = xh+xl, y' = -2y =
  yh+yl, q = qh+ql per coordinate, x^2 = x2h+x2m+x2l):
    lhsT rows: [xh]*3 [xh]*3 [xl]*3 [xl]*3 [1]*6 [x2h x2m x2l]
    rhs  rows: [yh]*3 [yl]*3 [yh]*3 [yl]*3 [qh]*3 [ql]*3 [1]*3
  Evacuation never materializes the distance matrix: per m-tile (128 samples),
  8192 columns stream through an 8-bank PSUM ring as two 2048-col groups that
  ScalarE converts to f16 (CC) and four 1024-col groups that VectorE consumes
  with fused running-min scans:
    tensor_tensor_scan(out, data0=PSUM_f32, data1=CC_f16, init=chain,
                       op0=min, op1=min)
  Each scan first-touches 1 PSUM + 1 CC element per cycle, and the chain's
  initial value threads the running min across the four scans, so the m-tile
  min falls out of the last scan's final column with no separate fold tree.
"""

import os

import numpy as np

import concourse.bass as bass
import concourse.tile as tile
from concourse import bacc, mybir, bass_utils

F32 = mybir.dt.float32
BF16 = mybir.dt.bfloat16
F16 = mybir.dt.float16
AX = mybir.AxisListType
OP = mybir.AluOpType
AF = mybir.ActivationFunctionType

BS = 8
S = 2048          # n_samples (grid points)
J = 8192          # n_points (preds/gts)
NM = S // 128     # 16 m-tiles
PACK = 8          # prep packing for rhs: [3*PACK, J/PACK]
JP = J // PACK    # 1024
GPACK = 8         # prep packing for grid: [3*GPACK, S/GPACK]
SP = S // GPACK   # 256

# lhsT/rhs row layout (K = 24)
#   rows 0-2   lhsT xh_c        rhs yh_c
#   rows 3-5   lhsT xh_c        rhs yl_c
#   rows 6-8   lhsT xl_c        rhs yh_c
#   rows 9-11  lhsT xl_c        rhs yl_c
#   rows 12-14 lhsT ones        rhs qh_c
#   rows 15-17 lhsT ones        rhs ql_c
#   rows 18-23 lhsT gqh_c/gql_c rhs ones     (x^2 = sum_c g_c^2 via contraction)
K = 24


def _load_pts(nc, sb, pts_dram, name):
    Y = sb.tile([3 * PACK, JP], F32, tag=f"y_{name}", name=f"Y{name}")
    nc.sync.dma_start(Y[:], pts_dram)
    return Y


def _build_rhs(nc, sb, Y, name, dma):
    """Build the [24, J] bf16 rhs from the loaded point set (packed [24, 1024])."""
    # q = y^2 per coordinate (ScalarE), yh = bf16(-2y) (ScalarE)
    SQ = sb.tile([3 * PACK, JP], F32, tag=f"sq_{name}")
    nc.scalar.activation(SQ[:], Y[:], AF.Square)
    YH = sb.tile([3 * PACK, JP], BF16, tag=f"yh_{name}")
    nc.scalar.activation(YH[:], Y[:], AF.Copy, scale=-2.0)
    # yl = (-2y) - yh (VectorE), qh = bf16(q) (ScalarE), ql = q - qh (VectorE)
    YL = sb.tile([3 * PACK, JP], BF16, tag=f"yl_{name}")
    nc.vector.scalar_tensor_tensor(YL[:], Y[:], -2.0, YH[:], op0=OP.mult, op1=OP.subtract)
    QH = sb.tile([3 * PACK, JP], BF16, tag=f"qh_{name}")
    nc.scalar.activation(QH[:], SQ[:], AF.Copy)
    QL = sb.tile([3 * PACK, JP], BF16, tag=f"ql_{name}")
    nc.vector.tensor_tensor(QL[:], SQ[:], QH[:], op=OP.subtract)
    ONESJ = sb.tile([3 * PACK, JP], BF16, tag=f"onesj_{name}")
    nc.gpsimd.memset(ONESJ[:], 1.0)

    RH = sb.tile([K, J], BF16, tag=f"rh_{name}")
    # packed [24, 1024] -> [3, 8192] row groups; AP iteration orders match.
    # ScalarE-sourced rows first so VectorE-dependent rows don't head-of-line
    # block the in-order DGE queue.
    for r0, src in ((18, ONESJ), (21, ONESJ), (0, YH), (6, YH), (12, QH),
                    (3, YL), (9, YL), (15, QL)):
        dma(RH[r0:r0 + 3, :], src[:])
    return RH


def _build_lhs(nc, sb, grid_dram):
    """Build the [24, S] bf16 lhsT from the packed grid [24, 256]."""
    GP = sb.tile([3 * GPACK, SP], F32, tag="gp")
    nc.sync.dma_start(GP[:], grid_dram)

    XH = sb.tile([3 * GPACK, SP], BF16, tag="xh")
    nc.scalar.activation(XH[:], GP[:], AF.Copy)
    XL = sb.tile([3 * GPACK, SP], BF16, tag="xl")
    nc.vector.tensor_tensor(XL[:], GP[:], XH[:], op=OP.subtract)
    # per-coord squares of the grid, split to bf16 pairs (x^2 via contraction)
    SQG = sb.tile([3 * GPACK, SP], F32, tag="sqg")
    nc.vector.tensor_tensor(SQG[:], GP[:], GP[:], op=OP.mult)
    GQH = sb.tile([3 * GPACK, SP], BF16, tag="gqh")
    nc.scalar.activation(GQH[:], SQG[:], AF.Copy)
    GQL = sb.tile([3 * GPACK, SP], BF16, tag="gql")
    nc.vector.tensor_tensor(GQL[:], SQG[:], GQH[:], op=OP.subtract)
    ONESS = sb.tile([3 * GPACK, SP], BF16, tag="oness")
    nc.gpsimd.memset(ONESS[:], 1.0)

    LH = sb.tile([K, S], BF16, tag="lh")
    for r0, src in ((0, XH), (6, XL), (12, ONESS), (18, GQH)):
        nc.sync.dma_start(LH[r0:r0 + 3, :], src[:])
    for r0, src in ((3, XH), (9, XL), (15, ONESS), (21, GQL)):
        nc.gpsimd.dma_start(LH[r0:r0 + 3, :], src[:])
    return LH


def _mtile(nc, wk, ps_a, ps_s, LH, RH, MINS, INF, m):
    """One m-tile: 4 act groups of 1024 (ScalarE -> f16 CC) and 4 independent
    1024-col running-min scans (VectorE) pairing fresh PSUM with CC.
    Both PSUM tags are double-buffered (8 banks total) so each group's
    matmuls prefill while the previous group is consumed."""
    if True:
        LHm = LH[:, m * 128:(m + 1) * 128]
        OB = wk.tile([128, 4096], F16, tag="so")
        for u in range(4):  # unit = [act 1024 | scan 1024], scans independent
            PA = ps_a.tile([128, 1024], F32, tag="pa")
            base = u * 2048
            for t in range(2):
                nc.tensor.matmul(PA[:, t * 512:(t + 1) * 512], LHm,
                                 RH[:, base + t * 512:base + (t + 1) * 512],
                                 start=True, stop=True)
            CC = wk.tile([128, 1024], F16, tag="cc", bufs=4)
            nc.scalar.activation(CC[:], PA[:], AF.Copy)
            PS = ps_s.tile([128, 1024], F32, tag="psc")
            for t in range(2):
                nc.tensor.matmul(PS[:, t * 512:(t + 1) * 512], LHm,
                                 RH[:, base + 1024 + t * 512:base + 1024 + (t + 1) * 512],
                                 start=True, stop=True)
            nc.vector.tensor_tensor_scan(OB[:, u * 1024:(u + 1) * 1024], PS[:], CC[:],
                                         INF[:], op0=OP.min, op1=OP.min)
        # m-tile min = min over the 4 independent scans' final columns
        nc.vector.tensor_reduce(MINS[:, m:m + 1], OB[:, 1023::1024], axis=AX.X, op=OP.min)


def _build_module():
    nc = bacc.Bacc("TRN2", target_bir_lowering=False, debug=False, num_devices=BS)
    grid_p = nc.dram_tensor("grid_p", [3 * GPACK, SP], F32, kind="ExternalInput").ap()
    preds_p = nc.dram_tensor("preds_p", [3 * PACK, JP], F32, kind="ExternalInput").ap()
    gts_p = nc.dram_tensor("gts_p", [3 * PACK, JP], F32, kind="ExternalInput").ap()
    out_d = nc.dram_tensor("out", [1, 1], F32, kind="ExternalOutput").ap()

    with tile.TileContext(nc) as tc:
        with tc.tile_pool(name="sb", bufs=1) as sb, \
             tc.tile_pool(name="wk", bufs=2) as wk, \
             tc.tile_pool(name="ps_a", bufs=2, space="PSUM") as ps_a, \
             tc.tile_pool(name="ps_s", bufs=2, space="PSUM") as ps_s:
            YP = _load_pts(nc, sb, preds_p, "p")
            YG = _load_pts(nc, sb, gts_p, "g")
            LH = _build_lhs(nc, sb, grid_p)
            RHP = _build_rhs(nc, sb, YP, "p", nc.sync.dma_start)
            RHG = _build_rhs(nc, sb, YG, "g", nc.gpsimd.dma_start)

            INF = sb.tile([128, 1], F32, tag="inf")
            nc.vector.memset(INF[:], 3.0e38)

            MINS_P = sb.tile([128, NM], F32, tag="minsp")
            MINS_G = sb.tile([128, NM], F32, tag="minsg")

            # d = sqrt(max(d^2, eps)) with one Newton refinement step
            def _distances(MINS, tag):
                D2 = sb.tile([128, NM], F32, tag=f"d2{tag}", name=f"d2{tag}")
                nc.vector.tensor_scalar_max(D2[:], MINS[:], 1e-12)
                D0 = sb.tile([128, NM], F32, tag=f"d0{tag}", name=f"d0{tag}")
                nc.scalar.activation(D0[:], D2[:], AF.Sqrt)
                R = sb.tile([128, NM], F32, tag=f"r{tag}", name=f"r{tag}")
                nc.vector.reciprocal(R[:], D0[:])
                D1 = sb.tile([128, NM], F32, tag=f"d1{tag}", name=f"d1{tag}")
                nc.vector.tensor_tensor(D1[:], D2[:], R[:], op=OP.mult)
                nc.vector.tensor_tensor(D1[:], D1[:], D0[:], op=OP.add)
                nc.vector.tensor_scalar_mul(D1[:], D1[:], 0.5)
                return D1

            for m in range(NM):
                _mtile(nc, wk, ps_a, ps_s, LH, RHP, MINS_P, INF, m)
            DP = _distances(MINS_P, "dp")
            for m in range(NM):
                _mtile(nc, wk, ps_a, ps_s, LH, RHG, MINS_G, INF, m)
            DG = _distances(MINS_G, "dg")

            # mean_s |dp - dg|
            DIFF = sb.tile([128, NM], F32, tag="diff")
            nc.vector.tensor_tensor(DIFF[:], DP[:], DG[:], op=OP.subtract)
            SROW = sb.tile([128, 1], F32, tag="srow")
            nc.vector.tensor_reduce(SROW[:], DIFF[:], axis=AX.X, op=OP.add,
                                    apply_absolute_value=True)
            ONE32 = sb.tile([128, 1], F32, tag="one32")
            nc.vector.memset(ONE32[:], 1.0)
            PGX = ps_a.tile([128, 1024], F32, tag="pa")
            TOT = PGX[0:1, 0:1]
            nc.tensor.matmul(TOT, ONE32[:], SROW[:], start=True, stop=True)
            OUT = sb.tile([1, 1], F32, tag="outsb")
            nc.scalar.activation(OUT[:], TOT, AF.Copy, scale=1.0 / float(S))
            nc.sync.dma_start(out_d, OUT[:])
    nc.compile()
    return nc


_NC = None


def _get_nc():
    global _NC
    if _NC is None:
        _NC = _build_module()
    return _NC


def _in_maps(gts, preds, grid_points):
    maps = []
    for b in range(BS):
        g = np.ascontiguousarray(grid_points[b], np.float32)
        maps.append({
            "grid_p": np.ascontiguousarray(g.T.reshape(3 * GPACK, SP)),
            "preds_p": np.ascontiguousarray(preds[b], np.float32).T.reshape(3 * PACK, JP).copy(),
            "gts_p": np.ascontiguousarray(gts[b], np.float32).T.reshape(3 * PACK, JP).copy(),
        })
    return maps


def kernel(gts, preds, grid_points, _trace=False, _trace_kwargs=None):
    nc = _get_nc()
    res = bass_utils.run_bass_kernel_spmd(
        nc, _in_maps(gts, preds, grid_points), core_ids=list(range(BS)),
        trace=_trace, **(_trace_kwargs or {}))
    out = np.array([res.results[b]["out"][0, 0] for b in range(BS)], np.float32)
    if _trace:
        return out, res
    return out


# revision 15
# speedup vs baseline: 1.0558x; 1.0013x over previous
"""Chamfer-augmented kernel for Trainium2 (8 NeuronCores, data-parallel over batch).

For each batch b and each grid sample s:
    mins[s]  = min_j ||grid_s - pred_j||
    mins2[s] = min_j ||grid_s - gt_j||
    out[b]   = mean_s |mins - mins2|

Per-core algorithm (batch b on core b):
  PSUM holds d^2(s,j) = x_s^2 + q_j - 2 x_s . y_j directly: a single K=21 bf16
  matmul per 512-col chunk using exact Karatsuba splits (x = xh+xl, y' = -2y =
  yh+yl, q = qh+ql per coordinate, x^2 = x2h+x2m+x2l):
    lhsT rows: [xh]*3 [xh]*3 [xl]*3 [xl]*3 [1]*6 [x2h x2m x2l]
    rhs  rows: [yh]*3 [yl]*3 [yh]*3 [yl]*3 [qh]*3 [ql]*3 [1]*3
  Evacuation never materializes the distance matrix: per m-tile (128 samples),
  8192 columns stream through an 8-bank PSUM ring as two 2048-col groups that
  ScalarE converts to f16 (CC) and four 1024-col groups that VectorE consumes
  with fused running-min scans:
    tensor_tensor_scan(out, data0=PSUM_f32, data1=CC_f16, init=chain,
                       op0=min, op1=min)
  Each scan first-touches 1 PSUM + 1 CC element per cycle, and the chain's
  initial value threads the running min across the four scans, so the m-tile
  min falls out of the last scan's final column with no separate fold tree.
"""

import os

import numpy as np

import concourse.bass as bass
import concourse.tile as tile
from concourse import bacc, mybir, bass_utils

F32 = mybir.dt.float32
BF16 = mybir.dt.bfloat16
F16 = mybir.dt.float16
AX = mybir.AxisListType
OP = mybir.AluOpType
AF = mybir.ActivationFunctionType

BS = 8
S = 2048          # n_samples (grid points)
J = 8192          # n_points (preds/gts)
NM = S // 128     # 16 m-tiles
PACK = 8          # prep packing for rhs: [3*PACK, J/PACK]
JP = J // PACK    # 1024
GPACK = 8         # prep packing for grid: [3*GPACK, S/GPACK]
SP = S // GPACK   # 256

# lhsT/rhs row layout (K = 24)
#   rows 0-2   lhsT xh_c        rhs yh_c
#   rows 3-5   lhsT xh_c        rhs yl_c
#   rows 6-8   lhsT xl_c        rhs yh_c
#   rows 9-11  lhsT xl_c        rhs yl_c
#   rows 12-14 lhsT ones        rhs qh_c
#   rows 15-17 lhsT ones        rhs ql_c
#   rows 18-23 lhsT gqh_c/gql_c rhs ones     (x^2 = sum_c g_c^2 via contraction)
K = 24


def _load_pts(nc, sb, pts_dram, name):
    Y = sb.tile([3 * PACK, JP], F32, tag=f"y_{name}", name=f"Y{name}")
    nc.sync.dma_start(Y[:], pts_dram)
    return Y


def _build_rhs(nc, sb, Y, name, dma):
    """Build the [24, J] bf16 rhs from the loaded point set (packed [24, 1024])."""
    # q = y^2 per coordinate (ScalarE), yh = bf16(-2y) (ScalarE)
    SQ = sb.tile([3 * PACK, JP], F32, tag=f"sq_{name}")
    nc.scalar.activation(SQ[:], Y[:], AF.Square)
    YH = sb.tile([3 * PACK, JP], BF16, tag=f"yh_{name}")
    nc.scalar.activation(YH[:], Y[:], AF.Copy, scale=-2.0)
    # yl = (-2y) - yh (VectorE), qh = bf16(q) (ScalarE), ql = q - qh (VectorE)
    YL = sb.tile([3 * PACK, JP], BF16, tag=f"yl_{name}")
    nc.vector.scalar_tensor_tensor(YL[:], Y[:], -2.0, YH[:], op0=OP.mult, op1=OP.subtract)
    QH = sb.tile([3 * PACK, JP], BF16, tag=f"qh_{name}")
    nc.scalar.activation(QH[:], SQ[:], AF.Copy)
    QL = sb.tile([3 * PACK, JP], BF16, tag=f"ql_{name}")
    nc.vector.tensor_tensor(QL[:], SQ[:], QH[:], op=OP.subtract)
    ONESJ = sb.tile([3 * PACK, JP], BF16, tag=f"onesj_{name}")
    nc.gpsimd.memset(ONESJ[:], 1.0)

    RH = sb.tile([K, J], BF16, tag=f"rh_{name}")
    # packed [24, 1024] -> [3, 8192] row groups; AP iteration orders match.
    # ScalarE-sourced rows first so VectorE-dependent rows don't head-of-line
    # block the in-order DGE queue.
    for r0, src in ((18, ONESJ), (21, ONESJ), (0, YH), (6, YH), (12, QH),
                    (3, YL), (9, YL), (15, QL)):
        dma(RH[r0:r0 + 3, :], src[:])
    return RH


def _build_lhs(nc, sb, grid_dram):
    """Build the [24, S] bf16 lhsT from the packed grid [24, 256]."""
    GP = sb.tile([3 * GPACK, SP], F32, tag="gp")
    nc.sync.dma_start(GP[:], grid_dram)

    XH = sb.tile([3 * GPACK, SP], BF16, tag="xh")
    nc.scalar.activation(XH[:], GP[:], AF.Copy)
    XL = sb.tile([3 * GPACK, SP], BF16, tag="xl")
    nc.vector.tensor_tensor(XL[:], GP[:], XH[:], op=OP.subtract)
    # per-coord squares of the grid, split to bf16 pairs (x^2 via contraction)
    SQG = sb.tile([3 * GPACK, SP], F32, tag="sqg")
    nc.vector.tensor_tensor(SQG[:], GP[:], GP[:], op=OP.mult)
    GQH = sb.tile([3 * GPACK, SP], BF16, tag="gqh")
    nc.scalar.activation(GQH[:], SQG[:], AF.Copy)
    GQL = sb.tile([3 * GPACK, SP], BF16, tag="gql")
    nc.vector.tensor_tensor(GQL[:], SQG[:], GQH[:], op=OP.subtract)
    ONESS = sb.tile([3 * GPACK, SP], BF16, tag="oness")
    nc.gpsimd.memset(ONESS[:], 1.0)

    LH = sb.tile([K, S], BF16, tag="lh")
    for r0, src in ((0, XH), (6, XL), (12, ONESS), (18, GQH)):
        nc.sync.dma_start(LH[r0:r0 + 3, :], src[:])
    for r0, src in ((3, XH), (9, XL), (15, ONESS), (21, GQL)):
        nc.gpsimd.dma_start(LH[r0:r0 + 3, :], src[:])
    return LH


def _mtile(nc, wk, ps_a, ps_s, LH, RH, MINS, INF, m):
    """One m-tile: 4 act groups of 1024 (ScalarE -> f16 CC) and 4 independent
    1024-col running-min scans (VectorE) pairing fresh PSUM with CC.
    Both PSUM tags are double-buffered (8 banks total) so each group's
    matmuls prefill while the previous group is consumed."""
    if True:
        LHm = LH[:, m * 128:(m + 1) * 128]
        OB = wk.tile([128, 4096], F16, tag="so")
        for u in range(4):  # unit = [act 1024 | scan 1024], scans independent
            PA = ps_a.tile([128, 1024], F32, tag="pa")
            base = u * 2048
            for t in range(2):
                nc.tensor.matmul(PA[:, t * 512:(t + 1) * 512], LHm,
                                 RH[:, base + t * 512:base + (t + 1) * 512],
                                 start=True, stop=True)
            CC = wk.tile([128, 1024], F16, tag="cc", bufs=4)
            nc.scalar.activation(CC[:], PA[:], AF.Copy)
            PS = ps_s.tile([128, 1024], F32, tag="psc")
            for t in range(2):
                nc.tensor.matmul(PS[:, t * 512:(t + 1) * 512], LHm,
                                 RH[:, base + 1024 + t * 512:base + 1024 + (t + 1) * 512],
                                 start=True, stop=True)
            nc.vector.tensor_tensor_scan(OB[:, u * 1024:(u + 1) * 1024], PS[:], CC[:],
                                         INF[:], op0=OP.min, op1=OP.min)
        # m-tile min = min over the 4 independent scans' final columns
        nc.vector.tensor_reduce(MINS[:, m:m + 1], OB[:, 1023::1024], axis=AX.X, op=OP.min)


def _build_module():
    nc = bacc.Bacc("TRN2", target_bir_lowering=False, debug=False, num_devices=BS)
    grid_p = nc.dram_tensor("grid_p", [3 * GPACK, SP], F32, kind="ExternalInput").ap()
    preds_p = nc.dram_tensor("preds_p", [3 * PACK, JP], F32, kind="ExternalInput").ap()
    gts_p = nc.dram_tensor("gts_p", [3 * PACK, JP], F32, kind="ExternalInput").ap()
    out_d = nc.dram_tensor("out", [1, 1], F32, kind="ExternalOutput").ap()

    with tile.TileContext(nc) as tc:
        with tc.tile_pool(name="sb", bufs=1) as sb, \
             tc.tile_pool(name="wk", bufs=2) as wk, \
             tc.tile_pool(name="ps_a", bufs=2, space="PSUM") as ps_a, \
             tc.tile_pool(name="ps_s", bufs=2, space="PSUM") as ps_s:
            YP = _load_pts(nc, sb, preds_p, "p")
            YG = _load_pts(nc, sb, gts_p, "g")
            LH = _build_lhs(nc, sb, grid_p)
            RHP = _build_rhs(nc, sb, YP, "p", nc.sync.dma_start)
            RHG = _build_rhs(nc, sb, YG, "g", nc.gpsimd.dma_start)

            INF = sb.tile([128, 1], F32, tag="inf")
            nc.vector.memset(INF[:], 3.0e38)

            MINS_P = sb.tile([128, NM], F32, tag="minsp")
            MINS_G = sb.tile([128, NM], F32, tag="minsg")

            # d = sqrt(max(d^2, eps)) with one Newton refinement step
            def _distances(MINS, tag):
                D2 = sb.tile([128, NM], F32, tag=f"d2{tag}", name=f"d2{tag}")
                nc.vector.tensor_scalar_max(D2[:], MINS[:], 1e-12)
                D0 = sb.tile([128, NM], F32, tag=f"d0{tag}", name=f"d0{tag}")
                nc.scalar.activation(D0[:], D2[:], AF.Sqrt)
                R = sb.tile([128, NM], F32, tag=f"r{tag}", name=f"r{tag}")
                nc.vector.reciprocal(R[:], D0[:])
                D1 = sb.tile([128, NM], F32, tag=f"d1{tag}", name=f"d1{tag}")
                nc.vector.tensor_tensor(D1[:], D2[:], R[:], op=OP.mult)
                nc.vector.tensor_tensor(D1[:], D1[:], D0[:], op=OP.add)
                return D1  # = 2*d; the 0.5 folds into the final mean scale

            for m in range(NM):
                _mtile(nc, wk, ps_a, ps_s, LH, RHP, MINS_P, INF, m)
            DP = _distances(MINS_P, "dp")
            for m in range(NM):
                _mtile(nc, wk, ps_a, ps_s, LH, RHG, MINS_G, INF, m)
            DG = _distances(MINS_G, "dg")

            # mean_s |dp - dg|
            DIFF = sb.tile([128, NM], F32, tag="diff")
            nc.vector.tensor_tensor(DIFF[:], DP[:], DG[:], op=OP.subtract)
            SROW = sb.tile([128, 1], F32, tag="srow")
            nc.vector.tensor_reduce(SROW[:], DIFF[:], axis=AX.X, op=OP.add,
                                    apply_absolute_value=True)
            ONE32 = sb.tile([128, 1], F32, tag="one32")
            nc.vector.memset(ONE32[:], 1.0)
            PGX = ps_a.tile([128, 1024], F32, tag="pa")
            TOT = PGX[0:1, 0:1]
            nc.tensor.matmul(TOT, ONE32[:], SROW[:], start=True, stop=True)
            OUT = sb.tile([1, 1], F32, tag="outsb")
            nc.scalar.activation(OUT[:], TOT, AF.Copy, scale=0.5 / float(S))
            nc.sync.dma_start(out_d, OUT[:])
    nc.compile()
    return nc


_NC = None


def _get_nc():
    global _NC
    if _NC is None:
        _NC = _build_module()
    return _NC


def _in_maps(gts, preds, grid_points):
    maps = []
    for b in range(BS):
        g = np.ascontiguousarray(grid_points[b], np.float32)
        maps.append({
            "grid_p": np.ascontiguousarray(g.T.reshape(3 * GPACK, SP)),
            "preds_p": np.ascontiguousarray(preds[b], np.float32).T.reshape(3 * PACK, JP).copy(),
            "gts_p": np.ascontiguousarray(gts[b], np.float32).T.reshape(3 * PACK, JP).copy(),
        })
    return maps


def kernel(gts, preds, grid_points, _trace=False, _trace_kwargs=None):
    nc = _get_nc()
    res = bass_utils.run_bass_kernel_spmd(
        nc, _in_maps(gts, preds, grid_points), core_ids=list(range(BS)),
        trace=_trace, **(_trace_kwargs or {}))
    out = np.array([res.results[b]["out"][0, 0] for b in range(BS)], np.float32)
    if _trace:
        return out, res
    return out


# revision 16
# speedup vs baseline: 1.0606x; 1.0046x over previous
"""Chamfer-augmented kernel for Trainium2 (8 NeuronCores, data-parallel over batch).

For each batch b and each grid sample s:
    mins[s]  = min_j ||grid_s - pred_j||
    mins2[s] = min_j ||grid_s - gt_j||
    out[b]   = mean_s |mins - mins2|

Per-core algorithm (batch b on core b):
  PSUM holds d^2(s,j) = x_s^2 + q_j - 2 x_s . y_j directly: a single K=21 bf16
  matmul per 512-col chunk using exact Karatsuba splits (x = xh+xl, y' = -2y =
  yh+yl, q = qh+ql per coordinate, x^2 = x2h+x2m+x2l):
    lhsT rows: [xh]*3 [xh]*3 [xl]*3 [xl]*3 [1]*6 [x2h x2m x2l]
    rhs  rows: [yh]*3 [yl]*3 [yh]*3 [yl]*3 [qh]*3 [ql]*3 [1]*3
  Evacuation never materializes the distance matrix: per m-tile (128 samples),
  8192 columns stream through an 8-bank PSUM ring as two 2048-col groups that
  ScalarE converts to f16 (CC) and four 1024-col groups that VectorE consumes
  with fused running-min scans:
    tensor_tensor_scan(out, data0=PSUM_f32, data1=CC_f16, init=chain,
                       op0=min, op1=min)
  Each scan first-touches 1 PSUM + 1 CC element per cycle, and the chain's
  initial value threads the running min across the four scans, so the m-tile
  min falls out of the last scan's final column with no separate fold tree.
"""

import os

import numpy as np

import concourse.bass as bass
import concourse.tile as tile
from concourse import bacc, mybir, bass_utils

F32 = mybir.dt.float32
BF16 = mybir.dt.bfloat16
F16 = mybir.dt.float16
AX = mybir.AxisListType
OP = mybir.AluOpType
AF = mybir.ActivationFunctionType

BS = 8
S = 2048          # n_samples (grid points)
J = 8192          # n_points (preds/gts)
NM = S // 128     # 16 m-tiles
PACK = 8          # prep packing for rhs: [3*PACK, J/PACK]
JP = J // PACK    # 1024
GPACK = 8         # prep packing for grid: [3*GPACK, S/GPACK]
SP = S // GPACK   # 256

# lhsT/rhs row layout (K = 24)
#   rows 0-2   lhsT xh_c        rhs yh_c
#   rows 3-5   lhsT xh_c        rhs yl_c
#   rows 6-8   lhsT xl_c        rhs yh_c
#   rows 9-11  lhsT xl_c        rhs yl_c
#   rows 12-14 lhsT ones        rhs qh_c
#   rows 15-17 lhsT ones        rhs ql_c
#   rows 18-23 lhsT gqh_c/gql_c rhs ones     (x^2 = sum_c g_c^2 via contraction)
K = 24


def _load_pts(nc, sb, pts_dram, name):
    Y = sb.tile([3 * PACK, JP], F32, tag=f"y_{name}", name=f"Y{name}")
    nc.sync.dma_start(Y[:], pts_dram)
    return Y


def _build_rhs(nc, sb, Y, name, dma):
    """Build the [24, J] bf16 rhs from the loaded point set (packed [24, 1024])."""
    # q = y^2 per coordinate (ScalarE), yh = bf16(-2y) (ScalarE)
    SQ = sb.tile([3 * PACK, JP], F32, tag=f"sq_{name}")
    nc.scalar.activation(SQ[:], Y[:], AF.Square)
    YH = sb.tile([3 * PACK, JP], BF16, tag=f"yh_{name}")
    nc.scalar.activation(YH[:], Y[:], AF.Copy, scale=-2.0)
    # yl = (-2y) - yh (VectorE), qh = bf16(q) (ScalarE), ql = q - qh (VectorE)
    YL = sb.tile([3 * PACK, JP], BF16, tag=f"yl_{name}")
    nc.vector.scalar_tensor_tensor(YL[:], Y[:], -2.0, YH[:], op0=OP.mult, op1=OP.subtract)
    QH = sb.tile([3 * PACK, JP], BF16, tag=f"qh_{name}")
    nc.scalar.activation(QH[:], SQ[:], AF.Copy)
    QL = sb.tile([3 * PACK, JP], BF16, tag=f"ql_{name}")
    nc.vector.tensor_tensor(QL[:], SQ[:], QH[:], op=OP.subtract)
    ONESJ = sb.tile([3 * PACK, JP], BF16, tag=f"onesj_{name}")
    nc.gpsimd.memset(ONESJ[:], 1.0)

    RH = sb.tile([K, J], BF16, tag=f"rh_{name}")
    # packed [24, 1024] -> [3, 8192] row groups; AP iteration orders match.
    # ScalarE-sourced rows first so VectorE-dependent rows don't head-of-line
    # block the in-order DGE queue.
    for r0, src in ((18, ONESJ), (21, ONESJ), (0, YH), (6, YH), (12, QH),
                    (3, YL), (9, YL), (15, QL)):
        dma(RH[r0:r0 + 3, :], src[:])
    return RH


def _build_lhs(nc, sb, grid_dram):
    """Build the [24, S] bf16 lhsT from the packed grid [24, 256]."""
    GP = sb.tile([3 * GPACK, SP], F32, tag="gp")
    nc.sync.dma_start(GP[:], grid_dram)

    XH = sb.tile([3 * GPACK, SP], BF16, tag="xh")
    nc.scalar.activation(XH[:], GP[:], AF.Copy)
    XL = sb.tile([3 * GPACK, SP], BF16, tag="xl")
    nc.vector.tensor_tensor(XL[:], GP[:], XH[:], op=OP.subtract)
    # per-coord squares of the grid, split to bf16 pairs (x^2 via contraction)
    SQG = sb.tile([3 * GPACK, SP], F32, tag="sqg")
    nc.vector.tensor_tensor(SQG[:], GP[:], GP[:], op=OP.mult)
    GQH = sb.tile([3 * GPACK, SP], BF16, tag="gqh")
    nc.scalar.activation(GQH[:], SQG[:], AF.Copy)
    GQL = sb.tile([3 * GPACK, SP], BF16, tag="gql")
    nc.vector.tensor_tensor(GQL[:], SQG[:], GQH[:], op=OP.subtract)
    ONESS = sb.tile([3 * GPACK, SP], BF16, tag="oness")
    nc.gpsimd.memset(ONESS[:], 1.0)

    LH = sb.tile([K, S], BF16, tag="lh")
    for r0, src in ((0, XH), (6, XL), (12, ONESS), (18, GQH)):
        nc.sync.dma_start(LH[r0:r0 + 3, :], src[:])
    for r0, src in ((3, XH), (9, XL), (15, ONESS), (21, GQL)):
        nc.gpsimd.dma_start(LH[r0:r0 + 3, :], src[:])
    return LH


def _mtile(nc, wk, ps_a, ps_s, LH, RH, MINS, INF, m):
    """One m-tile: 4 act groups of 1024 (ScalarE -> f16 CC) and 4 independent
    1024-col running-min scans (VectorE) pairing fresh PSUM with CC.
    Both PSUM tags are double-buffered (8 banks total) so each group's
    matmuls prefill while the previous group is consumed."""
    if True:
        LHm = LH[:, m * 128:(m + 1) * 128]
        OB = wk.tile([128, 4096], F16, tag="so", bufs=3)
        for u in range(4):  # unit = [act 1024 | scan 1024], scans independent
            PA = ps_a.tile([128, 1024], F32, tag="pa")
            base = u * 2048
            for t in range(2):
                nc.tensor.matmul(PA[:, t * 512:(t + 1) * 512], LHm,
                                 RH[:, base + t * 512:base + (t + 1) * 512],
                                 start=True, stop=True)
            CC = wk.tile([128, 1024], F16, tag="cc", bufs=6)
            nc.scalar.activation(CC[:], PA[:], AF.Copy)
            PS = ps_s.tile([128, 1024], F32, tag="psc")
            for t in range(2):
                nc.tensor.matmul(PS[:, t * 512:(t + 1) * 512], LHm,
                                 RH[:, base + 1024 + t * 512:base + 1024 + (t + 1) * 512],
                                 start=True, stop=True)
            nc.vector.tensor_tensor_scan(OB[:, u * 1024:(u + 1) * 1024], PS[:], CC[:],
                                         INF[:], op0=OP.min, op1=OP.min)
        # m-tile min = min over the 4 independent scans' final columns
        nc.vector.tensor_reduce(MINS[:, m:m + 1], OB[:, 1023::1024], axis=AX.X, op=OP.min)


def _build_module():
    nc = bacc.Bacc("TRN2", target_bir_lowering=False, debug=False, num_devices=BS)
    grid_p = nc.dram_tensor("grid_p", [3 * GPACK, SP], F32, kind="ExternalInput").ap()
    preds_p = nc.dram_tensor("preds_p", [3 * PACK, JP], F32, kind="ExternalInput").ap()
    gts_p = nc.dram_tensor("gts_p", [3 * PACK, JP], F32, kind="ExternalInput").ap()
    out_d = nc.dram_tensor("out", [1, 1], F32, kind="ExternalOutput").ap()

    with tile.TileContext(nc) as tc:
        with tc.tile_pool(name="sb", bufs=1) as sb, \
             tc.tile_pool(name="wk", bufs=2) as wk, \
             tc.tile_pool(name="ps_a", bufs=2, space="PSUM") as ps_a, \
             tc.tile_pool(name="ps_s", bufs=2, space="PSUM") as ps_s:
            YP = _load_pts(nc, sb, preds_p, "p")
            YG = _load_pts(nc, sb, gts_p, "g")
            LH = _build_lhs(nc, sb, grid_p)
            RHP = _build_rhs(nc, sb, YP, "p", nc.sync.dma_start)
            RHG = _build_rhs(nc, sb, YG, "g", nc.gpsimd.dma_start)

            INF = sb.tile([128, 1], F32, tag="inf")
            nc.vector.memset(INF[:], 3.0e38)

            MINS_P = sb.tile([128, NM], F32, tag="minsp")
            MINS_G = sb.tile([128, NM], F32, tag="minsg")

            # d = sqrt(max(d^2, eps)) with one Newton step; the two sets'
            # chains are issued alternating so per-op sem gaps overlap.
            def _distances2(MP, MG):
                D2P = sb.tile([128, NM], F32, tag="d2p", name="d2p")
                D2G = sb.tile([128, NM], F32, tag="d2g", name="d2g")
                nc.vector.tensor_scalar_max(D2P[:], MP[:], 1e-12)
                nc.vector.tensor_scalar_max(D2G[:], MG[:], 1e-12)
                D0P = sb.tile([128, NM], F32, tag="d0p", name="d0p")
                D0G = sb.tile([128, NM], F32, tag="d0g", name="d0g")
                nc.scalar.activation(D0P[:], D2P[:], AF.Sqrt)
                nc.scalar.activation(D0G[:], D2G[:], AF.Sqrt)
                RP = sb.tile([128, NM], F32, tag="rp", name="rp")
                RG = sb.tile([128, NM], F32, tag="rg", name="rg")
                nc.vector.reciprocal(RP[:], D0P[:])
                nc.vector.reciprocal(RG[:], D0G[:])
                D1P = sb.tile([128, NM], F32, tag="d1p", name="d1p")
                D1G = sb.tile([128, NM], F32, tag="d1g", name="d1g")
                nc.vector.tensor_tensor(D1P[:], D2P[:], RP[:], op=OP.mult)
                nc.vector.tensor_tensor(D1G[:], D2G[:], RG[:], op=OP.mult)
                nc.vector.tensor_tensor(D1P[:], D1P[:], D0P[:], op=OP.add)
                nc.vector.tensor_tensor(D1G[:], D1G[:], D0G[:], op=OP.add)
                return D1P, D1G  # = 2*d; 0.5 folds into the final mean scale

            for m in range(NM):
                _mtile(nc, wk, ps_a, ps_s, LH, RHP, MINS_P, INF, m)
            for m in range(NM):
                _mtile(nc, wk, ps_a, ps_s, LH, RHG, MINS_G, INF, m)
            DP, DG = _distances2(MINS_P, MINS_G)

            # mean_s |dp - dg|
            DIFF = sb.tile([128, NM], F32, tag="diff")
            nc.vector.tensor_tensor(DIFF[:], DP[:], DG[:], op=OP.subtract)
            SROW = sb.tile([128, 1], F32, tag="srow")
            nc.vector.tensor_reduce(SROW[:], DIFF[:], axis=AX.X, op=OP.add,
                                    apply_absolute_value=True)
            ONE32 = sb.tile([128, 1], F32, tag="one32")
            nc.vector.memset(ONE32[:], 1.0)
            PGX = ps_a.tile([128, 1024], F32, tag="pa")
            TOT = PGX[0:1, 0:1]
            nc.tensor.matmul(TOT, ONE32[:], SROW[:], start=True, stop=True)
            OUT = sb.tile([1, 1], F32, tag="outsb")
            nc.scalar.activation(OUT[:], TOT, AF.Copy, scale=0.5 / float(S))
            nc.sync.dma_start(out_d, OUT[:])
    nc.compile()
    return nc


_NC = None


def _get_nc():
    global _NC
    if _NC is None:
        _NC = _build_module()
    return _NC


def _in_maps(gts, preds, grid_points):
    maps = []
    for b in range(BS):
        g = np.ascontiguousarray(grid_points[b], np.float32)
        maps.append({
            "grid_p": np.ascontiguousarray(g.T.reshape(3 * GPACK, SP)),
            "preds_p": np.ascontiguousarray(preds[b], np.float32).T.reshape(3 * PACK, JP).copy(),
            "gts_p": np.ascontiguousarray(gts[b], np.float32).T.reshape(3 * PACK, JP).copy(),
        })
    return maps


def kernel(gts, preds, grid_points, _trace=False, _trace_kwargs=None):
    nc = _get_nc()
    res = bass_utils.run_bass_kernel_spmd(
        nc, _in_maps(gts, preds, grid_points), core_ids=list(range(BS)),
        trace=_trace, **(_trace_kwargs or {}))
    out = np.array([res.results[b]["out"][0, 0] for b in range(BS)], np.float32)
    if _trace:
        return out, res
    return out
